# revision 2
# baseline (speedup 1.0000x reference)
"""LoRA multi-head attention on 8 Trainium2 cores.

Sharding: core c handles batch n = c//4 and head-quarter g = c%4 (4 of 16
heads, feature slice [256g, 256g+256)).  LoRA adapters and the qk scale are
folded into the projection weights on the host.  Each core computes q/k/v
projections for its batch+heads, masked softmax attention (mask folded into
the logits matmul as an augmented K=65 contraction row), the unnormalized
attention output plus denominator (ones column appended to v), and a partial
output projection.  Host sums partial y over the 4 cores of each batch and
reconstructs the head-averaged attention weights from per-head unnormalized
bf16 scores shipped back in DRAM.
"""
import sys

sys.path.insert(0, "/opt/trn_rl_repo")

import numpy as np
import ml_dtypes

import concourse.bass as bass
import concourse.mybir as mybir
import concourse.tile as tile
from concourse import bacc
from concourse.bass_utils import run_bass_kernel_spmd
from concourse.masks import make_identity

F32 = mybir.dt.float32
F32R = mybir.dt.float32r
BF16 = mybir.dt.bfloat16

L = 2048          # sequence length
N = 2             # batch
E = 1024          # embed dim
H = 16            # total heads
HD = 64           # head dim
NH = 4            # heads per core
FS = NH * HD      # feature slice per core = 256
R = 8
SCALING = 4.0
MASK_NEG = -60000.0

KT = L // 128     # 16 key chunks of 128
QT = L // 512     # 4 query tiles of 512


def build_nc(trace_scopes=False):
    nc = bacc.Bacc("TRN2", target_bir_lowering=False, debug=False)

    xT_d = nc.dram_tensor("xT", [E, L], F32, kind="ExternalInput").ap()
    wqT_d = nc.dram_tensor("wqT", [E, FS], F32, kind="ExternalInput").ap()
    wkT_d = nc.dram_tensor("wkT", [E, FS], F32, kind="ExternalInput").ap()
    wvT_d = nc.dram_tensor("wvT", [E, FS], F32, kind="ExternalInput").ap()
    woT_d = nc.dram_tensor("woT", [FS, E], BF16, kind="ExternalInput").ap()
    bq_d = nc.dram_tensor("bq2", [128, 2], F32, kind="ExternalInput").ap()
    bk_d = nc.dram_tensor("bk2", [128, 2], F32, kind="ExternalInput").ap()
    bvb_d = nc.dram_tensor("bvb", [128, FS], F32, kind="ExternalInput").ap()
    mrow_d = nc.dram_tensor("mrow", [1, L], F32, kind="ExternalInput").ap()
    ones_d = nc.dram_tensor("onesr", [1, L], F32, kind="ExternalInput").ap()

    y_d = nc.dram_tensor("y", [L, E], F32, kind="ExternalOutput").ap()
    s_d = nc.dram_tensor("s_out", [NH, L, L], BF16, kind="ExternalOutput").ap()

    Exp = mybir.ActivationFunctionType.Exp
    Ident = mybir.ActivationFunctionType.Identity

    with tile.TileContext(nc) as tc:
        from contextlib import ExitStack

        with ExitStack() as ctx:
            persist = ctx.enter_context(tc.tile_pool(name="persist", bufs=1))
            psum_big = ctx.enter_context(
                tc.tile_pool(name="psb", bufs=3, space="PSUM"))
            psum_acc = ctx.enter_context(
                tc.tile_pool(name="psa", bufs=4, space="PSUM"))
            psum_tr = ctx.enter_context(
                tc.tile_pool(name="pst", bufs=1, space="PSUM"))

            # ---- persistent tiles -------------------------------------
            q_aug = [persist.tile([HD + 1, L], F32R, name=f"qaug{h}")
                     for h in range(NH)]
            k_aug = [persist.tile([HD + 1, L], F32R, name=f"kaug{h}")
                     for h in range(NH)]
            v_aug = [persist.tile([128, NH * (HD + 1)], BF16, name=f"vaug{t}")
                     for t in range(KT)]
            aoT = [persist.tile([128, L], BF16, name=f"aoT{j}")
                   for j in range(2)]
            ident = persist.tile([128, 128], F32, name="ident")
            bq_s = persist.tile([128, 2], F32, name="bq_s")
            bk_s = persist.tile([128, 2], F32, name="bk_s")
            bvb_s = persist.tile([128, FS], F32, name="bvb_s")

            make_identity(nc, ident[:])
            nc.sync.dma_start(bq_s[:], bq_d[:])
            nc.sync.dma_start(bk_s[:], bk_d[:])
            nc.sync.dma_start(bvb_s[:], bvb_d[:])
            for h in range(NH):
                nc.sync.dma_start(q_aug[h][HD:HD + 1, :],
                                  ones_d[:].bitcast(F32R))
                nc.sync.dma_start(k_aug[h][HD:HD + 1, :],
                                  mrow_d[:].bitcast(F32R))

            # ---- phase A/B: load x, weights; q/k/v projections --------
            with tc.tile_pool(name="early", bufs=1) as early:
                xs = [early.tile([128, L], F32R, name=f"xs{i}")
                      for i in range(8)]
                wq = [early.tile([128, FS], F32R, name=f"wq{i}")
                      for i in range(8)]
                wk = [early.tile([128, FS], F32R, name=f"wk{i}")
                      for i in range(8)]
                wv = [early.tile([128, FS], F32R, name=f"wv{i}")
                      for i in range(8)]
                for i in range(8):
                    sl = slice(i * 128, (i + 1) * 128)
                    nc.sync.dma_start(xs[i][:], xT_d[sl, :].bitcast(F32R))
                    nc.sync.dma_start(wq[i][:], wqT_d[sl, :].bitcast(F32R))
                    nc.sync.dma_start(wk[i][:], wkT_d[sl, :].bitcast(F32R))
                    nc.sync.dma_start(wv[i][:], wvT_d[sl, :].bitcast(F32R))

                # q/k projections: feature-major [FS, L]
                for which, w_t, b_t, dst in (
                        ("q", wq, bq_s, q_aug), ("k", wk, bk_s, k_aug)):
                    for ft in range(2):          # feat tile = head pair
                        for tt in range(QT):     # 512-token tiles
                            ps = psum_big.tile([128, 512], F32, tag="big",
                                               name=f"ps_{which}{ft}{tt}")
                            for kc in range(8):
                                nc.tensor.matmul(
                                    ps[:],
                                    w_t[kc][:, ft * 128:(ft + 1) * 128],
                                    xs[kc][:, tt * 512:(tt + 1) * 512],
                                    start=(kc == 0), stop=(kc == 7))
                            for sub in range(2):
                                h = 2 * ft + sub
                                nc.scalar.activation(
                                    dst[h][0:HD, tt * 512:(tt + 1) * 512],
                                    ps[sub * 64:sub * 64 + 64, :],
                                    Ident,
                                    bias=b_t[sub * 64:sub * 64 + 64,
                                             ft:ft + 1])

                # v projection: token-major [L, FS] -> v_aug blocks of 65
                for tt in range(KT):
                    ps = psum_big.tile([128, FS], F32, tag="big",
                                       name=f"ps_v{tt}")
                    for kc in range(8):
                        nc.tensor.matmul(
                            ps[:],
                            xs[kc][:, tt * 128:(tt + 1) * 128],
                            wv[kc][:],
                            start=(kc == 0), stop=(kc == 7))
                    # strided evict: 4 head blocks of 64 cols + bias
                    ps3 = ps[:].rearrange("p (h d) -> p h d", h=NH)
                    bv3 = bvb_s[:].rearrange("p (h d) -> p h d", h=NH)
                    va3 = v_aug[tt][:].rearrange(
                        "p (h e) -> p h e", h=NH)[:, :, 0:HD]
                    nc.vector.tensor_add(va3, ps3, bv3)
                    nc.vector.memset(
                        v_aug[tt][:].rearrange(
                            "p (h e) -> p h e", h=NH)[:, :, HD:HD + 1],
                        1.0)

            # ---- phase C: attention -----------------------------------
            with tc.tile_pool(name="late", bufs=1) as late, \
                 tc.tile_pool(name="spool", bufs=6) as spool, \
                 tc.tile_pool(name="aopool", bufs=4) as aopool, \
                 tc.tile_pool(name="rpool", bufs=4) as rpool, \
                 tc.tile_pool(name="ypool", bufs=4) as ypool:

                wo = [late.tile([128, E], BF16, name=f"wo{j}")
                      for j in range(2)]
                for j in range(2):
                    nc.sync.dma_start(
                        wo[j][:], woT_d[j * 128:(j + 1) * 128, :])

                for h in range(NH):
                    for qt in range(QT):
                        oacc = [psum_acc.tile([128, HD + 1], F32, tag="oacc",
                                              name=f"oacc{sq}")
                                for sq in range(4)]
                        for lk in range(KT):
                            ps_l = psum_big.tile([128, 512], F32, tag="big",
                                                 name="ps_l")
                            nc.tensor.matmul(
                                ps_l[:],
                                k_aug[h][:, lk * 128:(lk + 1) * 128],
                                q_aug[h][:, qt * 512:(qt + 1) * 512],
                                start=True, stop=True)
                            s_t = spool.tile([128, 512], BF16, tag="s",
                                             name="s_t")
                            nc.scalar.activation(s_t[:], ps_l[:], Exp)
                            nc.sync.dma_start(
                                s_d[h, lk * 128:(lk + 1) * 128,
                                    qt * 512:(qt + 1) * 512],
                                s_t[:])
                            for sq in range(4):
                                nc.tensor.matmul(
                                    oacc[sq][:],
                                    s_t[:, sq * 128:(sq + 1) * 128],
                                    v_aug[lk][:, h * (HD + 1):
                                              (h + 1) * (HD + 1)],
                                    start=(lk == 0), stop=(lk == KT - 1),
                                    skip_group_check=True)
                        for sq in range(4):
                            recip = rpool.tile([128, 1], F32, tag="r",
                                               name="recip")
                            nc.vector.reciprocal(
                                recip[:], oacc[sq][:, HD:HD + 1])
                            ao_n = aopool.tile([128, HD], F32, tag="ao",
                                               name="ao_n")
                            nc.vector.tensor_scalar_mul(
                                ao_n[:], oacc[sq][:, 0:HD], recip[:])
                            ps_t = psum_tr.tile([64, 128], F32, tag="tr",
                                                name="ps_t")
                            nc.tensor.transpose(ps_t[:], ao_n[:], ident[:])
                            j, po = h // 2, (h % 2) * 64
                            nc.scalar.activation(
                                aoT[j][po:po + 64,
                                       qt * 512 + sq * 128:
                                       qt * 512 + (sq + 1) * 128],
                                ps_t[:], Ident)

                # ---- phase D: output projection -----------------------
                for tt in range(KT):
                    for nh in range(2):
                        ps_y = psum_big.tile([128, 512], F32, tag="big",
                                             name="ps_y")
                        for j in range(2):
                            nc.tensor.matmul(
                                ps_y[:],
                                aoT[j][:, tt * 128:(tt + 1) * 128],
                                wo[j][:, nh * 512:(nh + 1) * 512],
                                start=(j == 0), stop=(j == 1))
                        y_sb = ypool.tile([128, 512], F32, tag="y",
                                          name="y_sb")
                        nc.vector.tensor_copy(y_sb[:], ps_y[:])
                        nc.sync.dma_start(
                            y_d[tt * 128:(tt + 1) * 128,
                                nh * 512:(nh + 1) * 512],
                            y_sb[:])

    nc.compile()
    return nc


_NC_CACHE = {}


def _get_nc():
    if "nc" not in _NC_CACHE:
        _NC_CACHE["nc"] = build_nc()
    return _NC_CACHE["nc"]


def make_in_maps(query, key_padding_mask, Wq, bq, Aq, Bq, Wk, bk, Ak, Bk,
                 Wv, bv, Av, Bv, Wo, bo, Ao, Bo):
    query = np.asarray(query, dtype=np.float32)
    mask = np.asarray(key_padding_mask)
    scale = HD ** -0.5

    Wq_eff = (np.asarray(Wq) + SCALING * np.asarray(Bq) @ np.asarray(Aq)) * scale
    Wk_eff = np.asarray(Wk) + SCALING * np.asarray(Bk) @ np.asarray(Ak)
    Wv_eff = np.asarray(Wv) + SCALING * np.asarray(Bv) @ np.asarray(Av)
    Wo_eff = np.asarray(Wo) + SCALING * np.asarray(Bo) @ np.asarray(Ao)
    bq_eff = np.asarray(bq) * scale

    ones_r = np.ones((1, L), dtype=np.float32)
    in_maps = []
    for c in range(8):
        n, g = c // 4, c % 4
        fs = slice(FS * g, FS * g + FS)
        xT = np.ascontiguousarray(query[:, n, :].T).astype(np.float32)
        madd = np.where(mask[n], np.float32(MASK_NEG),
                        np.float32(0.0)).astype(np.float32)[None, :]
        in_maps.append({
            "xT": xT,
            "wqT": np.ascontiguousarray(Wq_eff[fs].T).astype(np.float32),
            "wkT": np.ascontiguousarray(Wk_eff[fs].T).astype(np.float32),
            "wvT": np.ascontiguousarray(Wv_eff[fs].T).astype(np.float32),
            "woT": np.ascontiguousarray(Wo_eff[:, fs].T).astype(
                ml_dtypes.bfloat16),
            "bq2": np.ascontiguousarray(
                bq_eff[fs].reshape(2, 128).T).astype(np.float32),
            "bk2": np.ascontiguousarray(
                np.asarray(bk)[fs].reshape(2, 128).T).astype(np.float32),
            "bvb": np.broadcast_to(np.asarray(bv)[fs], (128, FS)).astype(
                np.float32).copy(),
            "mrow": np.ascontiguousarray(madd),
            "onesr": ones_r,
        })
    return in_maps


def assemble(results, bo):
    bo = np.asarray(bo, dtype=np.float32)
    attn_output = np.empty((L, N, E), dtype=np.float32)
    attn_weights = np.empty((L, N, L), dtype=np.float32)
    for n in range(N):
        y = np.zeros((L, E), dtype=np.float32)
        wsum = np.zeros((L, L), dtype=np.float32)   # [lk, lq]
        for g in range(4):
            r = results[4 * n + g]
            y += np.asarray(r["y"], dtype=np.float32)
            s = np.asarray(r["s_out"]).astype(np.float32)   # [NH, lk, lq]
            for h in range(NH):
                denom = s[h].sum(axis=0)                    # [lq]
                np.maximum(denom, np.float32(1e-37), out=denom)
                wsum += s[h] / denom[None, :]
        attn_output[:, n, :] = y + bo[None, :]
        attn_weights[:, n, :] = wsum.T / np.float32(H)
    return attn_output, attn_weights


def kernel(**inputs):
    nc = _get_nc()
    in_maps = make_in_maps(**inputs)
    res = run_bass_kernel_spmd(nc, in_maps, core_ids=list(range(8)))
    return assemble(res.results, inputs["bo"])


# revision 4
# speedup vs baseline: 1.1445x; 1.1445x over previous
"""LoRA multi-head attention on 8 Trainium2 cores.

Sharding: core c handles batch n = c//4 and head-quarter g = c%4 (4 of 16
heads, feature slice [256g, 256g+256)).  LoRA adapters and the qk scale are
folded into the projection weights on the host.  Each core computes q/k/v
projections for its batch+heads, masked softmax attention (mask folded into
the logits matmul as an augmented K=65 contraction row), the unnormalized
attention output plus denominator (ones column appended to v), and a partial
output projection.  Host sums partial y over the 4 cores of each batch and
reconstructs the head-averaged attention weights from per-head unnormalized
bf16 scores shipped back in DRAM.
"""
import sys

sys.path.insert(0, "/opt/trn_rl_repo")

import numpy as np
import ml_dtypes

import concourse.bass as bass
import concourse.mybir as mybir
import concourse.tile as tile
from concourse import bacc
from concourse.bass_utils import run_bass_kernel_spmd
from concourse.masks import make_identity

F32 = mybir.dt.float32
F32R = mybir.dt.float32r
BF16 = mybir.dt.bfloat16

L = 2048          # sequence length
N = 2             # batch
E = 1024          # embed dim
H = 16            # total heads
HD = 64           # head dim
NH = 4            # heads per core
FS = NH * HD      # feature slice per core = 256
R = 8
SCALING = 4.0
MASK_NEG = -60000.0

KT = L // 128     # 16 key chunks of 128
QT = L // 512     # 4 query tiles of 512


def build_nc(trace_scopes=False):
    nc = bacc.Bacc("TRN2", target_bir_lowering=False, debug=False)

    xT_d = nc.dram_tensor("xT", [E, L], F32, kind="ExternalInput").ap()
    wqT_d = nc.dram_tensor("wqT", [E, FS], F32, kind="ExternalInput").ap()
    wkT_d = nc.dram_tensor("wkT", [E, FS], F32, kind="ExternalInput").ap()
    wvT_d = nc.dram_tensor("wvT", [E, FS], F32, kind="ExternalInput").ap()
    woT_d = nc.dram_tensor("woT", [FS, E], BF16, kind="ExternalInput").ap()
    bq_d = nc.dram_tensor("bq2", [128, 2], F32, kind="ExternalInput").ap()
    bk_d = nc.dram_tensor("bk2", [128, 2], F32, kind="ExternalInput").ap()
    bvb_d = nc.dram_tensor("bvb", [128, FS], F32, kind="ExternalInput").ap()
    mrow_d = nc.dram_tensor("mrow", [1, L], F32, kind="ExternalInput").ap()
    ones_d = nc.dram_tensor("onesr", [1, L], F32, kind="ExternalInput").ap()

    y_d = nc.dram_tensor("y", [L, E], F32, kind="ExternalOutput").ap()
    s_d = nc.dram_tensor("s_out", [NH, L, L], BF16, kind="ExternalOutput").ap()

    Exp = mybir.ActivationFunctionType.Exp
    Ident = mybir.ActivationFunctionType.Identity

    with tile.TileContext(nc) as tc:
        from contextlib import ExitStack

        with ExitStack() as ctx:
            persist = ctx.enter_context(tc.tile_pool(name="persist", bufs=1))
            psum_big = ctx.enter_context(
                tc.tile_pool(name="psb", bufs=7, space="PSUM"))
            psum_tr = ctx.enter_context(
                tc.tile_pool(name="pst", bufs=1, space="PSUM"))

            # ---- persistent tiles -------------------------------------
            q_aug = [persist.tile([HD + 1, L], F32R, name=f"qaug{h}")
                     for h in range(NH)]
            k_aug = [persist.tile([HD + 1, L], F32R, name=f"kaug{h}")
                     for h in range(NH)]
            v_aug = [persist.tile([128, NH * (HD + 1)], BF16, name=f"vaug{t}")
                     for t in range(KT)]
            aoT = [persist.tile([128, L], BF16, name=f"aoT{j}")
                   for j in range(2)]
            ident = persist.tile([128, 128], F32, name="ident")
            bq_s = persist.tile([128, 2], F32, name="bq_s")
            bk_s = persist.tile([128, 2], F32, name="bk_s")
            bvb_s = persist.tile([128, FS], F32, name="bvb_s")

            make_identity(nc, ident[:])
            nc.sync.dma_start(bq_s[:], bq_d[:])
            nc.sync.dma_start(bk_s[:], bk_d[:])
            nc.sync.dma_start(bvb_s[:], bvb_d[:])
            for h in range(NH):
                nc.sync.dma_start(q_aug[h][HD:HD + 1, :],
                                  ones_d[:].bitcast(F32R))
                nc.sync.dma_start(k_aug[h][HD:HD + 1, :],
                                  mrow_d[:].bitcast(F32R))

            # ---- phase A/B: load x, weights; q/k/v projections --------
            with tc.tile_pool(name="early", bufs=1) as early:
                xs = [early.tile([128, L], F32R, name=f"xs{i}")
                      for i in range(8)]
                wq = [early.tile([128, FS], F32R, name=f"wq{i}")
                      for i in range(8)]
                wk = [early.tile([128, FS], F32R, name=f"wk{i}")
                      for i in range(8)]
                wv = [early.tile([128, FS], F32R, name=f"wv{i}")
                      for i in range(8)]
                for i in range(8):
                    sl = slice(i * 128, (i + 1) * 128)
                    nc.sync.dma_start(xs[i][:], xT_d[sl, :].bitcast(F32R))
                    nc.sync.dma_start(wq[i][:], wqT_d[sl, :].bitcast(F32R))
                    nc.sync.dma_start(wk[i][:], wkT_d[sl, :].bitcast(F32R))
                    nc.sync.dma_start(wv[i][:], wvT_d[sl, :].bitcast(F32R))

                # q/k projections: feature-major [FS, L]
                for which, w_t, b_t, dst in (
                        ("q", wq, bq_s, q_aug), ("k", wk, bk_s, k_aug)):
                    for ft in range(2):          # feat tile = head pair
                        for tt in range(QT):     # 512-token tiles
                            ps = psum_big.tile([128, 512], F32, tag="big",
                                               name=f"ps_{which}{ft}{tt}")
                            for kc in range(8):
                                nc.tensor.matmul(
                                    ps[:],
                                    w_t[kc][:, ft * 128:(ft + 1) * 128],
                                    xs[kc][:, tt * 512:(tt + 1) * 512],
                                    start=(kc == 0), stop=(kc == 7))
                            for sub in range(2):
                                h = 2 * ft + sub
                                nc.scalar.activation(
                                    dst[h][0:HD, tt * 512:(tt + 1) * 512],
                                    ps[sub * 64:sub * 64 + 64, :],
                                    Ident,
                                    bias=b_t[sub * 64:sub * 64 + 64,
                                             ft:ft + 1])

                # v projection: token-major [L, FS] -> v_aug blocks of 65
                for tt in range(KT):
                    ps = psum_big.tile([128, FS], F32, tag="big",
                                       name=f"ps_v{tt}")
                    for kc in range(8):
                        nc.tensor.matmul(
                            ps[:],
                            xs[kc][:, tt * 128:(tt + 1) * 128],
                            wv[kc][:],
                            start=(kc == 0), stop=(kc == 7))
                    # strided evict: 4 head blocks of 64 cols + bias
                    ps3 = ps[:].rearrange("p (h d) -> p h d", h=NH)
                    bv3 = bvb_s[:].rearrange("p (h d) -> p h d", h=NH)
                    va3 = v_aug[tt][:].rearrange(
                        "p (h e) -> p h e", h=NH)[:, :, 0:HD]
                    nc.vector.tensor_add(va3, ps3, bv3)
                    nc.vector.memset(
                        v_aug[tt][:].rearrange(
                            "p (h e) -> p h e", h=NH)[:, :, HD:HD + 1],
                        1.0)

            # ---- phase C: attention -----------------------------------
            with tc.tile_pool(name="late", bufs=1) as late, \
                 tc.tile_pool(name="spool", bufs=34) as spool, \
                 tc.tile_pool(name="aopool", bufs=4) as aopool, \
                 tc.tile_pool(name="rpool", bufs=4) as rpool, \
                 tc.tile_pool(name="ypool", bufs=4) as ypool:

                wo = [late.tile([128, E], BF16, name=f"wo{j}")
                      for j in range(2)]
                for j in range(2):
                    nc.sync.dma_start(
                        wo[j][:], woT_d[j * 128:(j + 1) * 128, :])

                blocks = [(h, qt) for h in range(NH) for qt in range(QT)]
                s_store = {}

                def emit_logits(i):
                    h, qt = blocks[i]
                    tiles = []
                    for lk in range(KT):
                        ps_l = psum_big.tile([128, 512], F32, tag="big",
                                             name="ps_l")
                        nc.tensor.matmul(
                            ps_l[:],
                            k_aug[h][:, lk * 128:(lk + 1) * 128],
                            q_aug[h][:, qt * 512:(qt + 1) * 512],
                            start=True, stop=True)
                        s_t = spool.tile([128, 512], BF16, tag="s",
                                         name="s_t")
                        nc.scalar.activation(s_t[:], ps_l[:], Exp)
                        nc.sync.dma_start(
                            s_d[h, lk * 128:(lk + 1) * 128,
                                qt * 512:(qt + 1) * 512],
                            s_t[:])
                        tiles.append(s_t)
                    s_store[i] = tiles

                def emit_out(i):
                    h, qt = blocks[i]
                    tiles = s_store.pop(i)
                    oacc = [psum_big.tile([128, HD + 1], F32, tag="big",
                                           name=f"oacc{sq}")
                            for sq in range(4)]
                    for lk in range(KT):
                        for sq in range(4):
                            nc.tensor.matmul(
                                oacc[sq][:],
                                tiles[lk][:, sq * 128:(sq + 1) * 128],
                                v_aug[lk][:, h * (HD + 1):
                                          (h + 1) * (HD + 1)],
                                start=(lk == 0), stop=(lk == KT - 1),
                                skip_group_check=True)
                    for sq in range(4):
                        recip = rpool.tile([128, 1], F32, tag="r",
                                           name="recip")
                        nc.vector.reciprocal(
                            recip[:], oacc[sq][:, HD:HD + 1])
                        ao_n = aopool.tile([128, HD], F32, tag="ao",
                                           name="ao_n")
                        nc.vector.tensor_scalar_mul(
                            ao_n[:], oacc[sq][:, 0:HD], recip[:])
                        ps_t = psum_tr.tile([64, 128], F32, tag="tr",
                                            name="ps_t")
                        nc.tensor.transpose(ps_t[:], ao_n[:], ident[:])
                        j, po = h // 2, (h % 2) * 64
                        nc.vector.tensor_copy(
                            aoT[j][po:po + 64,
                                   qt * 512 + sq * 128:
                                   qt * 512 + (sq + 1) * 128],
                            ps_t[:])

                emit_logits(0)
                for i in range(1, len(blocks)):
                    emit_logits(i)
                    emit_out(i - 1)
                emit_out(len(blocks) - 1)

                # ---- phase D: output projection -----------------------
                for tt in range(KT):
                    for nh in range(2):
                        ps_y = psum_big.tile([128, 512], F32, tag="big",
                                             name="ps_y")
                        for j in range(2):
                            nc.tensor.matmul(
                                ps_y[:],
                                aoT[j][:, tt * 128:(tt + 1) * 128],
                                wo[j][:, nh * 512:(nh + 1) * 512],
                                start=(j == 0), stop=(j == 1))
                        y_sb = ypool.tile([128, 512], F32, tag="y",
                                          name="y_sb")
                        nc.vector.tensor_copy(y_sb[:], ps_y[:])
                        nc.sync.dma_start(
                            y_d[tt * 128:(tt + 1) * 128,
                                nh * 512:(nh + 1) * 512],
                            y_sb[:])

    nc.compile()
    return nc


_NC_CACHE = {}


def _get_nc():
    if "nc" not in _NC_CACHE:
        _NC_CACHE["nc"] = build_nc()
    return _NC_CACHE["nc"]


def make_in_maps(query, key_padding_mask, Wq, bq, Aq, Bq, Wk, bk, Ak, Bk,
                 Wv, bv, Av, Bv, Wo, bo, Ao, Bo):
    query = np.asarray(query, dtype=np.float32)
    mask = np.asarray(key_padding_mask)
    scale = HD ** -0.5

    Wq_eff = (np.asarray(Wq) + SCALING * np.asarray(Bq) @ np.asarray(Aq)) * scale
    Wk_eff = np.asarray(Wk) + SCALING * np.asarray(Bk) @ np.asarray(Ak)
    Wv_eff = np.asarray(Wv) + SCALING * np.asarray(Bv) @ np.asarray(Av)
    Wo_eff = np.asarray(Wo) + SCALING * np.asarray(Bo) @ np.asarray(Ao)
    bq_eff = np.asarray(bq) * scale

    ones_r = np.ones((1, L), dtype=np.float32)
    in_maps = []
    for c in range(8):
        n, g = c // 4, c % 4
        fs = slice(FS * g, FS * g + FS)
        xT = np.ascontiguousarray(query[:, n, :].T).astype(np.float32)
        madd = np.where(mask[n], np.float32(MASK_NEG),
                        np.float32(0.0)).astype(np.float32)[None, :]
        in_maps.append({
            "xT": xT,
            "wqT": np.ascontiguousarray(Wq_eff[fs].T).astype(np.float32),
            "wkT": np.ascontiguousarray(Wk_eff[fs].T).astype(np.float32),
            "wvT": np.ascontiguousarray(Wv_eff[fs].T).astype(np.float32),
            "woT": np.ascontiguousarray(Wo_eff[:, fs].T).astype(
                ml_dtypes.bfloat16),
            "bq2": np.ascontiguousarray(
                bq_eff[fs].reshape(2, 128).T).astype(np.float32),
            "bk2": np.ascontiguousarray(
                np.asarray(bk)[fs].reshape(2, 128).T).astype(np.float32),
            "bvb": np.broadcast_to(np.asarray(bv)[fs], (128, FS)).astype(
                np.float32).copy(),
            "mrow": np.ascontiguousarray(madd),
            "onesr": ones_r,
        })
    return in_maps


def assemble(results, bo):
    bo = np.asarray(bo, dtype=np.float32)
    attn_output = np.empty((L, N, E), dtype=np.float32)
    attn_weights = np.empty((L, N, L), dtype=np.float32)
    for n in range(N):
        y = np.zeros((L, E), dtype=np.float32)
        wsum = np.zeros((L, L), dtype=np.float32)   # [lk, lq]
        for g in range(4):
            r = results[4 * n + g]
            y += np.asarray(r["y"], dtype=np.float32)
            s = np.asarray(r["s_out"]).astype(np.float32)   # [NH, lk, lq]
            for h in range(NH):
                denom = s[h].sum(axis=0)                    # [lq]
                np.maximum(denom, np.float32(1e-37), out=denom)
                wsum += s[h] / denom[None, :]
        attn_output[:, n, :] = y + bo[None, :]
        attn_weights[:, n, :] = wsum.T / np.float32(H)
    return attn_output, attn_weights


def kernel(**inputs):
    nc = _get_nc()
    in_maps = make_in_maps(**inputs)
    res = run_bass_kernel_spmd(nc, in_maps, core_ids=list(range(8)))
    return assemble(res.results, inputs["bo"])


# revision 5
# speedup vs baseline: 1.3675x; 1.1948x over previous
"""LoRA multi-head attention on 8 Trainium2 cores.

Sharding: core c handles batch n = c//4 and head-quarter g = c%4 (4 of 16
heads, feature slice [256g, 256g+256)).  LoRA adapters and the qk scale are
folded into the projection weights on the host.  Each core computes q/k/v
projections for its batch+heads, masked softmax attention (mask folded into
the logits matmul as an augmented K=65 contraction row), the unnormalized
attention output plus denominator (ones column appended to v), and a partial
output projection.  Host sums partial y over the 4 cores of each batch and
reconstructs the head-averaged attention weights from per-head unnormalized
bf16 scores shipped back in DRAM.
"""
import sys

sys.path.insert(0, "/opt/trn_rl_repo")

import numpy as np
import ml_dtypes

import concourse.bass as bass
import concourse.mybir as mybir
import concourse.tile as tile
from concourse import bacc
from concourse.bass_utils import run_bass_kernel_spmd
from concourse.masks import make_identity

F32 = mybir.dt.float32
F32R = mybir.dt.float32r
BF16 = mybir.dt.bfloat16

L = 2048          # sequence length
N = 2             # batch
E = 1024          # embed dim
H = 16            # total heads
HD = 64           # head dim
NH = 4            # heads per core
FS = NH * HD      # feature slice per core = 256
R = 8
SCALING = 4.0
MASK_NEG = -60000.0

KT = L // 128     # 16 key chunks of 128
QT = L // 512     # 4 query tiles of 512


def build_nc(trace_scopes=False):
    nc = bacc.Bacc("TRN2", target_bir_lowering=False, debug=False)

    xT_d = nc.dram_tensor("xT", [E, L], F32, kind="ExternalInput").ap()
    wqT_d = nc.dram_tensor("wqT", [E, FS], F32, kind="ExternalInput").ap()
    wkT_d = nc.dram_tensor("wkT", [E, FS], F32, kind="ExternalInput").ap()
    wvT_d = nc.dram_tensor("wvT", [E, FS], F32, kind="ExternalInput").ap()
    woT_d = nc.dram_tensor("woT", [FS, E], BF16, kind="ExternalInput").ap()
    bq_d = nc.dram_tensor("bq2", [128, 2], F32, kind="ExternalInput").ap()
    bk_d = nc.dram_tensor("bk2", [128, 2], F32, kind="ExternalInput").ap()
    bvb_d = nc.dram_tensor("bvb", [128, FS], F32, kind="ExternalInput").ap()
    mrow_d = nc.dram_tensor("mrow", [1, L], F32, kind="ExternalInput").ap()
    ones_d = nc.dram_tensor("onesr", [1, L], F32, kind="ExternalInput").ap()

    y_d = nc.dram_tensor("y", [L, E], F32, kind="ExternalOutput").ap()
    s_d = nc.dram_tensor("s_out", [NH, QT, KT // 2, 128, 1024], BF16,
                         kind="ExternalOutput").ap()

    Exp = mybir.ActivationFunctionType.Exp
    Ident = mybir.ActivationFunctionType.Identity

    with tile.TileContext(nc) as tc:
        from contextlib import ExitStack

        with ExitStack() as ctx:
            persist = ctx.enter_context(tc.tile_pool(name="persist", bufs=1))
            psum_wide = ctx.enter_context(
                tc.tile_pool(name="psw", bufs=2, space="PSUM"))
            psum_big = ctx.enter_context(
                tc.tile_pool(name="psb", bufs=4, space="PSUM"))

            # ---- persistent tiles -------------------------------------
            q_aug = [persist.tile([HD + 1, L], F32R, name=f"qaug{h}")
                     for h in range(NH)]
            k_aug = [persist.tile([HD + 1, L], F32R, name=f"kaug{h}")
                     for h in range(NH)]
            v_aug = [persist.tile([128, NH * (HD + 1)], BF16, name=f"vaug{t}")
                     for t in range(KT)]
            aoT = [persist.tile([128, L], BF16, name=f"aoT{j}")
                   for j in range(2)]
            ident = persist.tile([128, 128], F32, name="ident")
            bq_s = persist.tile([128, 2], F32, name="bq_s")
            bk_s = persist.tile([128, 2], F32, name="bk_s")
            bvb_s = persist.tile([128, FS], F32, name="bvb_s")

            make_identity(nc, ident[:])
            nc.sync.dma_start(bq_s[:], bq_d[:])
            nc.sync.dma_start(bk_s[:], bk_d[:])
            nc.sync.dma_start(bvb_s[:], bvb_d[:])
            for h in range(NH):
                nc.sync.dma_start(q_aug[h][HD:HD + 1, :],
                                  ones_d[:].bitcast(F32R))
                nc.sync.dma_start(k_aug[h][HD:HD + 1, :],
                                  mrow_d[:].bitcast(F32R))

            # ---- phase A/B: load x, weights; q/k/v projections --------
            with tc.tile_pool(name="early", bufs=1) as early:
                xs = [early.tile([128, L], F32R, name=f"xs{i}")
                      for i in range(8)]
                wq = [early.tile([128, FS], F32R, name=f"wq{i}")
                      for i in range(8)]
                wk = [early.tile([128, FS], F32R, name=f"wk{i}")
                      for i in range(8)]
                wv = [early.tile([128, FS], F32R, name=f"wv{i}")
                      for i in range(8)]
                for i in range(8):
                    sl = slice(i * 128, (i + 1) * 128)
                    nc.sync.dma_start(xs[i][:], xT_d[sl, :].bitcast(F32R))
                    nc.sync.dma_start(wq[i][:], wqT_d[sl, :].bitcast(F32R))
                    nc.sync.dma_start(wk[i][:], wkT_d[sl, :].bitcast(F32R))
                    nc.sync.dma_start(wv[i][:], wvT_d[sl, :].bitcast(F32R))

                # q/k projections: feature-major [FS, L]
                for which, w_t, b_t, dst in (
                        ("q", wq, bq_s, q_aug), ("k", wk, bk_s, k_aug)):
                    for ft in range(2):          # feat tile = head pair
                        for tt in range(QT):     # 512-token tiles
                            ps = psum_big.tile([128, 512], F32, tag="big",
                                               name=f"ps_{which}{ft}{tt}")
                            for kc in range(8):
                                nc.tensor.matmul(
                                    ps[:],
                                    w_t[kc][:, ft * 128:(ft + 1) * 128],
                                    xs[kc][:, tt * 512:(tt + 1) * 512],
                                    start=(kc == 0), stop=(kc == 7))
                            for sub in range(2):
                                h = 2 * ft + sub
                                nc.scalar.activation(
                                    dst[h][0:HD, tt * 512:(tt + 1) * 512],
                                    ps[sub * 64:sub * 64 + 64, :],
                                    Ident,
                                    bias=b_t[sub * 64:sub * 64 + 64,
                                             ft:ft + 1])

                # v projection: token-major [L, FS] -> v_aug blocks of 65
                for tt in range(KT):
                    ps = psum_big.tile([128, FS], F32, tag="big",
                                       name=f"ps_v{tt}")
                    for kc in range(8):
                        nc.tensor.matmul(
                            ps[:],
                            xs[kc][:, tt * 128:(tt + 1) * 128],
                            wv[kc][:],
                            start=(kc == 0), stop=(kc == 7))
                    # strided evict: 4 head blocks of 64 cols + bias
                    ps3 = ps[:].rearrange("p (h d) -> p h d", h=NH)
                    bv3 = bvb_s[:].rearrange("p (h d) -> p h d", h=NH)
                    va3 = v_aug[tt][:].rearrange(
                        "p (h e) -> p h e", h=NH)[:, :, 0:HD]
                    nc.vector.tensor_add(va3, ps3, bv3)
                    nc.vector.memset(
                        v_aug[tt][:].rearrange(
                            "p (h e) -> p h e", h=NH)[:, :, HD:HD + 1],
                        1.0)

            # ---- phase C: attention -----------------------------------
            with tc.tile_pool(name="late", bufs=1) as late, \
                 tc.tile_pool(name="spool", bufs=18) as spool, \
                 tc.tile_pool(name="aopool", bufs=4) as aopool, \
                 tc.tile_pool(name="rpool", bufs=4) as rpool, \
                 tc.tile_pool(name="ypool", bufs=4) as ypool:

                wo = [late.tile([128, E], BF16, name=f"wo{j}")
                      for j in range(2)]
                for j in range(2):
                    nc.sync.dma_start(
                        wo[j][:], woT_d[j * 128:(j + 1) * 128, :])

                blocks = [(h, qt) for h in range(NH) for qt in range(QT)]
                s_store = {}
                NB = len(blocks)

                def emit_logits_pair(i, j):
                    # pair j covers lk chunks 2j, 2j+1 for block i
                    h, qt = blocks[i]
                    ps_l = psum_wide.tile([128, 1024], F32, tag="wide",
                                          name="ps_l")
                    for par in range(2):
                        lk = 2 * j + par
                        nc.tensor.matmul(
                            ps_l[:, par * 512:(par + 1) * 512],
                            k_aug[h][:, lk * 128:(lk + 1) * 128],
                            q_aug[h][:, qt * 512:(qt + 1) * 512],
                            start=True, stop=True)
                    s_t = spool.tile([128, 1024], BF16, tag="s", name="s_t")
                    nc.scalar.activation(s_t[:], ps_l[:], Exp)
                    nc.sync.dma_start(s_d[h, qt, j], s_t[:])
                    s_store.setdefault(i, []).append(s_t)

                def emit_out_pair(i, j, oacc):
                    h, qt = blocks[i]
                    tiles = s_store[i]
                    for par in range(2):
                        lk = 2 * j + par
                        for sq in range(4):
                            nc.tensor.matmul(
                                oacc[sq][:],
                                tiles[j][:, par * 512 + sq * 128:
                                         par * 512 + (sq + 1) * 128],
                                v_aug[lk][:, h * (HD + 1):
                                          (h + 1) * (HD + 1)],
                                start=(lk == 0), stop=(lk == KT - 1),
                                skip_group_check=True)

                def emit_out_tail(i):
                    h, qt = blocks[i]
                    oacc = s_store.pop(i + NB)   # stashed accs
                    for sq in range(4):
                        recip = rpool.tile([128, 1], F32, tag="r",
                                           name="recip")
                        nc.vector.reciprocal(
                            recip[:], oacc[sq][:, HD:HD + 1])
                        ao_n = aopool.tile([128, HD], F32, tag="ao",
                                           name="ao_n")
                        nc.vector.tensor_scalar_mul(
                            ao_n[:], oacc[sq][:, 0:HD], recip[:])
                        ps_t = psum_big.tile([64, 128], F32, tag="big",
                                             name="ps_t")
                        nc.tensor.transpose(ps_t[:], ao_n[:], ident[:])
                        j2, po = h // 2, (h % 2) * 64
                        nc.vector.tensor_copy(
                            aoT[j2][po:po + 64,
                                    qt * 512 + sq * 128:
                                    qt * 512 + (sq + 1) * 128],
                            ps_t[:])

                for j in range(KT // 2):
                    emit_logits_pair(0, j)
                for i in range(NB):
                    if i < NB - 1:
                        oacc = [psum_big.tile([128, HD + 1], F32, tag="big",
                                              name=f"oacc{sq}")
                                for sq in range(4)]
                        s_store[i + NB] = oacc
                        for j in range(KT // 2):
                            emit_logits_pair(i + 1, j)
                            emit_out_pair(i, j, oacc)
                        s_store.pop(i)
                        emit_out_tail(i)
                    else:
                        oacc = [psum_big.tile([128, HD + 1], F32, tag="big",
                                              name=f"oacc{sq}")
                                for sq in range(4)]
                        s_store[i + NB] = oacc
                        for j in range(KT // 2):
                            emit_out_pair(i, j, oacc)
                        s_store.pop(i)
                        emit_out_tail(i)

                # ---- phase D: output projection -----------------------
                for tt in range(KT):
                    for nh in range(2):
                        ps_y = psum_big.tile([128, 512], F32, tag="big",
                                             name="ps_y")
                        for j in range(2):
                            nc.tensor.matmul(
                                ps_y[:],
                                aoT[j][:, tt * 128:(tt + 1) * 128],
                                wo[j][:, nh * 512:(nh + 1) * 512],
                                start=(j == 0), stop=(j == 1))
                        y_sb = ypool.tile([128, 512], F32, tag="y",
                                          name="y_sb")
                        nc.vector.tensor_copy(y_sb[:], ps_y[:])
                        nc.sync.dma_start(
                            y_d[tt * 128:(tt + 1) * 128,
                                nh * 512:(nh + 1) * 512],
                            y_sb[:])

    nc.compile()
    return nc


_NC_CACHE = {}


def _get_nc():
    if "nc" not in _NC_CACHE:
        _NC_CACHE["nc"] = build_nc()
    return _NC_CACHE["nc"]


def make_in_maps(query, key_padding_mask, Wq, bq, Aq, Bq, Wk, bk, Ak, Bk,
                 Wv, bv, Av, Bv, Wo, bo, Ao, Bo):
    query = np.asarray(query, dtype=np.float32)
    mask = np.asarray(key_padding_mask)
    scale = HD ** -0.5

    Wq_eff = (np.asarray(Wq) + SCALING * np.asarray(Bq) @ np.asarray(Aq)) * scale
    Wk_eff = np.asarray(Wk) + SCALING * np.asarray(Bk) @ np.asarray(Ak)
    Wv_eff = np.asarray(Wv) + SCALING * np.asarray(Bv) @ np.asarray(Av)
    Wo_eff = np.asarray(Wo) + SCALING * np.asarray(Bo) @ np.asarray(Ao)
    bq_eff = np.asarray(bq) * scale

    ones_r = np.ones((1, L), dtype=np.float32)
    in_maps = []
    for c in range(8):
        n, g = c // 4, c % 4
        fs = slice(FS * g, FS * g + FS)
        xT = np.ascontiguousarray(query[:, n, :].T).astype(np.float32)
        madd = np.where(mask[n], np.float32(MASK_NEG),
                        np.float32(0.0)).astype(np.float32)[None, :]
        in_maps.append({
            "xT": xT,
            "wqT": np.ascontiguousarray(Wq_eff[fs].T).astype(np.float32),
            "wkT": np.ascontiguousarray(Wk_eff[fs].T).astype(np.float32),
            "wvT": np.ascontiguousarray(Wv_eff[fs].T).astype(np.float32),
            "woT": np.ascontiguousarray(Wo_eff[:, fs].T).astype(
                ml_dtypes.bfloat16),
            "bq2": np.ascontiguousarray(
                bq_eff[fs].reshape(2, 128).T).astype(np.float32),
            "bk2": np.ascontiguousarray(
                np.asarray(bk)[fs].reshape(2, 128).T).astype(np.float32),
            "bvb": np.broadcast_to(np.asarray(bv)[fs], (128, FS)).astype(
                np.float32).copy(),
            "mrow": np.ascontiguousarray(madd),
            "onesr": ones_r,
        })
    return in_maps


def assemble(results, bo):
    bo = np.asarray(bo, dtype=np.float32)
    attn_output = np.empty((L, N, E), dtype=np.float32)
    attn_weights = np.empty((L, N, L), dtype=np.float32)
    for n in range(N):
        y = np.zeros((L, E), dtype=np.float32)
        wsum = np.zeros((L, L), dtype=np.float32)   # [lk, lq]
        for g in range(4):
            r = results[4 * n + g]
            y += np.asarray(r["y"], dtype=np.float32)
            arr = np.asarray(r["s_out"])
            s = arr.reshape(NH, QT, 8, 128, 2, 512).transpose(
                0, 2, 4, 3, 1, 5).reshape(NH, L, L).astype(np.float32)
            for h in range(NH):
                denom = s[h].sum(axis=0)                    # [lq]
                np.maximum(denom, np.float32(1e-37), out=denom)
                wsum += s[h] / denom[None, :]
        attn_output[:, n, :] = y + bo[None, :]
        attn_weights[:, n, :] = wsum.T / np.float32(H)
    return attn_output, attn_weights


def kernel(**inputs):
    nc = _get_nc()
    in_maps = make_in_maps(**inputs)
    res = run_bass_kernel_spmd(nc, in_maps, core_ids=list(range(8)))
    return assemble(res.results, inputs["bo"])
